# revision 30
# baseline (speedup 1.0000x reference)
"""Trainium2 Bass kernel for nn_CausalAttention (N=4096, 8 heads, DH=32).

Strategy: head-parallel across 8 NeuronCores (1 head per core).

Final (row-pipelined): the Scalar/ACT engine (exp over the causal score area,
~74K elems/partition) is the hard throughput floor (~62us/core), so the
kernel is organized to keep ACT busy from ~3us on and hide everything else
under it:
  - Software pipeline: DMA slice s -> project slice s -> process query-row s
    (scores + exp + PV for query block 512s..512s+512, k-tiles 0..4s+3).
    Row s only needs projections from slices <= s, so exp starts right
    after the first 1MB of input lands instead of after all projections.
  - Exp batching: scores grouped 3 k-tiles per PSUM tile [128, 1536]
    (double-buffered, 6 banks) -> 51 ACTIVATE calls instead of 72.
  - Score matmuls 4-way row-packed (K=32 quadrants of the PE array) via a
    kT4 layout: band u of partitions holds k-tile 4g+u; qT is 4x
    partition-replicated so all bands see the same queries.
  - All DMA on the Sync HWDGE queue (gpsimd software-DGE is ~100x slower;
    Scalar triggers would eat exp throughput). Inputs host-cast to bf16
    (halves HBM traffic); first k+q slice fused into one transfer; later
    slices in 1024-col interleaved chunks for descriptor efficiency.
  - 15 dummy matmuls at start warm the PE clock (HAM 1.2->2.4GHz)
    during the initial DMA wait.
  - Max-free softmax: P^T = exp(S / sqrt(32)); strict-causal 0/1 mask
    applied post-exp on the diagonal tiles (exp never overflows; the
    reference's -10000 masking underflows to exactly 0 in f32).
  - Softmax denominator via a ones column appended to V (PV lhsT [128, 33]);
    P^T and V in bf16 (f32 accumulate). Normalization: colsum -> reshape
    DMA -> reciprocal -> K=1 matmul broadcast -> one tensor_mul. Output
    stays in O^T layout [32, 4096] per core; host reshapes.
"""

import math

import numpy as np
import ml_dtypes

import concourse.bass as bass
import concourse.mybir as mybir
from concourse import bacc
from concourse.tile import TileContext
from concourse.bass_utils import run_bass_kernel_spmd

# Problem constants (hardcoded per harness contract).
B, CQ, CK, CH, NH, H, W = 1, 256, 256, 256, 8, 64, 64
DH = CH // NH            # 32
N = H * W                # 4096
QB = 512                 # queries per row (query block)
NQB = N // QB            # 8 rows
KT = 128                 # keys per k-tile
NKT = N // KT            # 32
EG = 3                   # k-tiles per exp/PSUM group (3 banks per tile)
SCALE = 1.0 / math.sqrt(DH)

F32 = mybir.dt.float32
F32R = mybir.dt.float32r
BF16 = mybir.dt.bfloat16

_CACHED_NC = None


def _build():
    nc = bacc.Bacc("TRN2", target_bir_lowering=False, debug=False, num_devices=1)

    qin_d = nc.dram_tensor("qin", [CQ, N], BF16, kind="ExternalInput")
    kin_d = nc.dram_tensor("kin", [CK, N], BF16, kind="ExternalInput")
    # fused first slice: kq0[(c p), t*512+n] = (kin if t==0 else qin)[:, n]
    kq0_d = nc.dram_tensor("kq0", [CQ, 2 * QB], BF16, kind="ExternalInput")
    wq_d = nc.dram_tensor("wqt", [CQ, 128], BF16, kind="ExternalInput")
    wk_d = nc.dram_tensor("wkt", [CK, 128], BF16, kind="ExternalInput")
    wv_d = nc.dram_tensor("wvt", [CK, DH], BF16, kind="ExternalInput")
    # bias_all[:, 0]=bq, [:, 1]=bk, [:, 2:2+4*DH]=bv (4x tiled)
    ball_d = nc.dram_tensor("ball", [128, 2 + 4 * DH], F32, kind="ExternalInput")
    out_d = nc.dram_tensor("out", [DH, N], F32, kind="ExternalOutput")

    # Strict-causal mask window: tm[kk, j] = 1.0 iff kk < j - 384; the
    # [*, 384:512] slice gives mask[kk, qq] = (kk < qq) for the 128-wide
    # diagonal window.
    tm_np = (np.arange(128)[:, None] < (np.arange(512)[None, :] - 384)).astype(
        ml_dtypes.bfloat16
    )
    tm_d = nc.inline_tensor(tm_np, name="tmask")
    ones_d = nc.inline_tensor(np.ones((1, DH), dtype=np.float32), name="onesd")

    with TileContext(nc) as tc:
        with (
            tc.tile_pool(name="constp", bufs=1) as constp,
            tc.tile_pool(name="bigp", bufs=1) as bigp,
            tc.tile_pool(name="workp", bufs=4) as workp,
            tc.tile_pool(name="spool", bufs=2, space="PSUM") as spool,
            tc.tile_pool(name="xpool", bufs=1, space="PSUM") as xpool,
            tc.tile_pool(name="opool", bufs=1, space="PSUM") as opool,
        ):
            # ---- PE warmup: dummy matmuls during the initial DMA wait so
            # HAM un-throttles the PE clock (1.2->2.4 GHz) before real work
            warm_sb = constp.tile([128, 512], BF16, name="warm_sb")
            nc.vector.memset(warm_sb[:], 0.0)
            warm_ps = xpool.tile([128, 512], F32, name="warm_ps", tag="x")
            for _ in range(15):
                nc.tensor.matmul(
                    warm_ps[:], warm_sb[:, 0:128], warm_sb[:], start=True, stop=True
                )

            # ---- weights/constants + inputs, all on the Sync HWDGE queue,
            # ordered so the projection chain unblocks earliest
            kin_sb = bigp.tile([128, 2, N], BF16, name="kin_sb")
            qin_sb = bigp.tile([128, 2, N], BF16, name="qin_sb")
            kin_ap = kin_d.ap().rearrange("(c p) n -> p c n", p=128)
            qin_ap = qin_d.ap().rearrange("(c p) n -> p c n", p=128)

            # fused slice 0 of kin+qin in one transfer (biggest first-exp win)
            sl0 = slice(0, QB)
            kq0_ap = kq0_d.ap().rearrange("(c p) (t n) -> p c t n", p=128, t=2)
            nc.sync.dma_start(kin_sb[:, :, sl0], kq0_ap[:, :, 0, :])
            nc.sync.dma_start(qin_sb[:, :, sl0], kq0_ap[:, :, 1, :])
            wk_sb = constp.tile([128, 2, 128], BF16, name="wk_sb")
            nc.sync.dma_start(
                wk_sb[:], wk_d.ap().rearrange("(c p) m -> p c m", p=128)
            )
            wq_sb = constp.tile([128, 2, 128], BF16, name="wq_sb")
            nc.sync.dma_start(
                wq_sb[:], wq_d.ap().rearrange("(c p) m -> p c m", p=128)
            )
            ball_sb = constp.tile([128, 2 + 4 * DH], F32, name="ball_sb")
            bq_sb = ball_sb[:, 0:1]
            bk_sb = ball_sb[:, 1:2]
            bv_sb = ball_sb[:, 2 : 2 + 4 * DH].rearrange("p (a b) -> p a b", a=4)
            # only bq/bk gate the first quad; bv + wv can follow slice 1
            nc.sync.dma_start(ball_sb[:, 0:2], ball_d.ap()[:, 0:2])
            # interleave kin/qin chunks so early rows unblock soonest
            nc.sync.dma_start(kin_sb[:, :, 512:1024], kin_ap[:, :, 512:1024])
            nc.sync.dma_start(qin_sb[:, :, 512:1024], qin_ap[:, :, 512:1024])
            nc.sync.dma_start(
                ball_sb[:, 2 : 2 + 4 * DH], ball_d.ap()[:, 2 : 2 + 4 * DH]
            )
            wv_sb = constp.tile([128, 2, DH], BF16, name="wv_sb")
            nc.sync.dma_start(
                wv_sb[:], wv_d.ap().rearrange("(c p) m -> p c m", p=128)
            )
            nc.sync.dma_start(kin_sb[:, :, 1024:1536], kin_ap[:, :, 1024:1536])
            nc.sync.dma_start(qin_sb[:, :, 1024:1536], qin_ap[:, :, 1024:1536])
            tm_sb = constp.tile([128, 512], BF16, name="tm_sb")
            nc.sync.dma_start(tm_sb[:], tm_d.ap())
            nc.sync.dma_start(kin_sb[:, :, 1536:2560], kin_ap[:, :, 1536:2560])
            nc.sync.dma_start(qin_sb[:, :, 1536:2560], qin_ap[:, :, 1536:2560])
            ones_sb = constp.tile([1, DH], F32, name="ones_sb")
            nc.sync.dma_start(ones_sb[:], ones_d.ap())
            nc.sync.dma_start(kin_sb[:, :, 2560:4096], kin_ap[:, :, 2560:4096])
            nc.sync.dma_start(qin_sb[:, :, 2560:4096], qin_ap[:, :, 2560:4096])

            # ---- persistent projected tensors ----
            # kT4[32u+d, 128g+kk] = k^T[d, 128*(4g+u)+kk]  (4-way row packing)
            kT4 = bigp.tile([128, (NKT // 4) * 128], BF16, name="kT4")
            # qT[32u+d, q] = q^T[d, q] for u=0..3 (4x replicated on partitions)
            qT = bigp.tile([128, N], BF16, name="qT")
            # v_all[kk, t, :DH] = v[128t+kk, :]; col DH is the ones column
            v_all = bigp.tile([128, NKT, 48], BF16, name="v_all")
            nc.vector.memset(v_all[:, :, DH : DH + 1], 1.0)

            def emit_kq_proj(s):
                """Project slice s: k-tiles 4s..4s+3 into kT4 + qT block s."""
                ksl = slice(QB * s, QB * (s + 1))
                pj = xpool.tile([128, 512], F32, name="pj", tag="x")
                for ch in range(2):
                    nc.tensor.matmul(
                        pj[:],
                        wk_sb[:, ch, :],
                        kin_sb[:, ch, ksl],
                        start=(ch == 0),
                        stop=(ch == 1),
                    )
                for ci in range(4):
                    # k-tile j = 4s+ci -> band u=ci, column group g=s.
                    # Early slices: do the adds on ScalarE (idle pre-exp,
                    # identity is in the exp table set) so they run in
                    # parallel with the qT add on DVE.
                    dst = kT4[32 * ci : 32 * ci + 32, 128 * s : 128 * s + 128]
                    srcp = pj[32 * ci : 32 * ci + 32, 128 * ci : 128 * ci + 128]
                    bias = bk_sb[32 * ci : 32 * ci + 32, :]
                    if s <= 1:
                        nc.scalar.activation(
                            dst,
                            srcp,
                            mybir.ActivationFunctionType.Identity,
                            bias=bias,
                        )
                    else:
                        nc.vector.tensor_scalar_add(dst, srcp, bias)
                pj = xpool.tile([128, 512], F32, name="pj", tag="x")
                for ch in range(2):
                    nc.tensor.matmul(
                        pj[:],
                        wq_sb[:, ch, :],
                        qin_sb[:, ch, ksl],
                        start=(ch == 0),
                        stop=(ch == 1),
                    )
                nc.vector.tensor_scalar_add(qT[:, ksl], pj[:], bq_sb[:])

            def emit_v_proj(s):
                """Project slice s into v_all tiles 4s..4s+3 (one shared
                psum bank for all 4 tiles, one combined bias-add)."""
                pj = xpool.tile([128, 4, DH], F32, name="pj", tag="x")
                for ti in range(4):
                    t = 4 * s + ti
                    nsl = slice(128 * t, 128 * (t + 1))
                    for ch in range(2):
                        nc.tensor.matmul(
                            pj[:, ti, :],
                            kin_sb[:, ch, nsl],
                            wv_sb[:, ch, :],
                            start=(ch == 0),
                            stop=(ch == 1),
                        )
                nc.vector.tensor_add(
                    v_all[:, 4 * s : 4 * s + 4, 0:DH], pj[:], bv_sb[:]
                )

            stage_q = []  # deferred tail stages, advanced one per exp group

            def tail_a(st):
                o_ps = st["o_ps"]
                o_sb = workp.tile([DH, 512], F32, name="o_sb")
                nc.vector.tensor_copy(o_sb[:], o_ps[0:DH, :])
                cs_sb = workp.tile([1, 512], F32, name="cs_sb")
                # +1e-30 keeps q=0 (fully masked row) at 0 instead of NaN
                nc.vector.tensor_scalar_add(cs_sb[:], o_ps[DH : DH + 1, :], 1e-30)
                st.update(o_sb=o_sb, cs_sb=cs_sb)

            def tail_b(st):
                # ~18-bit reciprocal, no DMA round trips
                csr = workp.tile([1, 512], F32, name="csr")
                nc.vector.reciprocal_approx_fast(csr[:], st["cs_sb"][:])
                st.update(csr=csr)

            def tail_c(st):
                qb = st["qb"]
                rep_ps = xpool.tile([DH, 512], F32, name="rep_ps", tag="x")
                nc.tensor.matmul(
                    rep_ps[:], ones_sb[:], st["csr"][:], start=True, stop=True
                )
                out_sb = workp.tile([DH, 512], F32, name="out_sb")
                nc.vector.tensor_mul(out_sb[:], st["o_sb"][:], rep_ps[:])
                nc.sync.dma_start(
                    out_d.ap()[:, 512 * qb : 512 * (qb + 1)], out_sb[:]
                )

            # Global deferred queues: PV work for a finished exp group is
            # flushed lazily (possibly during the NEXT row's quads) so the
            # in-order PE stream never makes ACT wait at a row boundary.
            pends = []  # (row_state, g, nsub, p_sb, row_final)

            def flush_one():
                st, g, nsub, p_sb, row_final = pends.pop(0)
                nlast = st["nkt"] - 1
                for u in range(nsub):
                    j = EG * g + u
                    nc.tensor.matmul(
                        st["o_ps"][0 : DH + 1, :],
                        v_all[:, j, 0 : DH + 1],
                        p_sb[:, 512 * u : 512 * (u + 1)],
                        start=st["first"][0],
                        stop=(j == nlast),
                        skip_group_check=True,
                    )
                    st["first"][0] = False
                if row_final:
                    tail_a(st)
                    stage_q.append(lambda st=st: tail_b(st))
                    stage_q.append(lambda: None)
                    stage_q.append(lambda: None)
                    stage_q.append(lambda st=st: tail_c(st))

            def emit_row(s):
                """Queries 512s..512s+512 vs k-tiles 0..4s+3."""
                nkt_q = 4 * (s + 1)
                ngr = (nkt_q + EG - 1) // EG
                o_ps = opool.tile([128, 512], F32, name="o_ps", tag="o")
                st = {"qb": s, "o_ps": o_ps, "first": [True, True], "nkt": nkt_q}
                groups = []

                def finish_group(g, nsub):
                    """Exp + mask + queue PV for a fully-written group."""
                    s_ps = groups[g][0]
                    p_sb = workp.tile(
                        [128, EG * 512], BF16, name="p_sb", tag="p", bufs=8
                    )
                    nc.scalar.activation(
                        p_sb[:, 0 : 512 * nsub],
                        s_ps[:, 0 : 512 * nsub],
                        mybir.ActivationFunctionType.Exp,
                        scale=SCALE,
                    )
                    for u in range(nsub):
                        j = EG * g + u
                        o = 128 * j - 512 * s
                        if o > 0:  # zero the fully-masked prefix (stale exp)
                            nc.vector.memset(p_sb[:, 512 * u : 512 * u + o], 0.0)
                        if o >= 0:  # strict-causal mask on the diagonal window
                            nc.vector.tensor_mul(
                                p_sb[:, 512 * u + o : 512 * u + o + 128],
                                p_sb[:, 512 * u + o : 512 * u + o + 128],
                                tm_sb[:, 384:512],
                            )
                    pends.append((st, g, nsub, p_sb, g == ngr - 1))
                    # near a row boundary let PV queue up so the next row's
                    # first quad isn't stuck behind it in the PE stream
                    # (final row excepted: drain fast so the tail starts)
                    cap = 4 if (g >= ngr - 2 and s < NQB - 1) else 2
                    if len(pends) > cap:
                        flush_one()
                    if stage_q:
                        stage_q.pop(0)()

                ndone = 0  # exp groups emitted
                for m in range(s + 1):  # score quads
                    # ensure psum groups for tiles 4m..4m+3 exist
                    while len(groups) * EG < 4 * (m + 1):
                        g = len(groups)
                        nsub = min(EG, nkt_q - EG * g)
                        t = spool.tile([128, EG * 512], F32, name="s_ps", tag="s")
                        groups.append((t, nsub))
                    for u in range(4):
                        j = 4 * m + u
                        g, slot = j // EG, j % EG
                        o = max(0, 128 * j - 512 * s)
                        nc.tensor.matmul(
                            groups[g][0][:, 512 * slot + o : 512 * (slot + 1)],
                            kT4[32 * u : 32 * u + 32, 128 * m : 128 * m + 128],
                            qT[32 * u : 32 * u + 32, 512 * s + o : 512 * (s + 1)],
                            start=True,
                            stop=True,
                            tile_position=(32 * u, 0),
                        )
                    # next slice's projections early, under this row's exps
                    if s == 0 and m == 0:
                        emit_v_proj(0)
                    if s + 1 < NQB:
                        if m == 0:
                            emit_kq_proj(s + 1)
                        if m == min(1, s):
                            emit_v_proj(s + 1)
                    # emit exp for every fully-written group
                    while (ndone + 1) * EG <= 4 * (m + 1) or (
                        m == s and ndone < ngr
                    ):
                        finish_group(ndone, groups[ndone][1])
                        ndone += 1

            emit_kq_proj(0)
            for s in range(NQB):
                emit_row(s)
            while pends:
                flush_one()
            while stage_q:
                stage_q.pop(0)()

    nc.finalize()
    return nc


def _get_nc():
    global _CACHED_NC
    if _CACHED_NC is None:
        _CACHED_NC = _build()
    return _CACHED_NC


def _prep_in_maps(inputs):
    f = lambda a: np.ascontiguousarray(np.asarray(a, dtype=np.float32))
    query = f(inputs["query"]).reshape(CQ, N)
    key_feat = f(inputs["key_feat"]).reshape(CK, N)

    def wnorm(v, g):
        v = f(v)
        g = f(g)
        return g[:, None] * v / np.linalg.norm(v, axis=1, keepdims=True)

    wq = wnorm(inputs["vq"], inputs["gq"])
    wk = wnorm(inputs["vk"], inputs["gk"])
    wv = wnorm(inputs["vv"], inputs["gv"])
    bq, bk, bv = f(inputs["bq"]), f(inputs["bk"]), f(inputs["bv"])

    bf = lambda a: np.ascontiguousarray(a).astype(ml_dtypes.bfloat16)
    query_bf = bf(query)
    key_bf = bf(key_feat)
    kq0 = np.concatenate([key_feat[:, :QB], query[:, :QB]], axis=1)
    in_maps = []
    for c in range(NH):
        rows = slice(DH * c, DH * (c + 1))
        in_maps.append(
            {
                "qin": query_bf,
                "kin": key_bf,
                "kq0": bf(kq0),
                "wqt": bf(np.tile(wq[rows].T, (1, 4))),
                "wkt": bf(np.tile(wk[rows].T, (1, 4))),
                "wvt": bf(wv[rows].T),
                "ball": np.ascontiguousarray(
                    np.concatenate(
                        [
                            np.tile(bq[rows], 4)[:, None],
                            np.tile(bk[rows], 4)[:, None],
                            np.tile(np.tile(bv[rows], 4)[None, :], (128, 1)),
                        ],
                        axis=1,
                    )
                ),
            }
        )
    return in_maps


def _run(inputs, trace=False, **kwargs):
    nc = _get_nc()
    in_maps = _prep_in_maps(inputs)
    res = None
    for attempt in range(3):
        try:
            res = run_bass_kernel_spmd(
                nc, in_maps, core_ids=list(range(NH)), trace=trace, **kwargs
            )
            break
        except Exception:
            if attempt == 2:
                raise

    out = np.empty((B, CH, H, W), dtype=np.float32)
    for c in range(NH):
        oc = res.results[c]["out"]  # [DH, N] (O^T layout)
        out[0, DH * c : DH * (c + 1)] = oc.reshape(DH, H, W)
    return out, res


def kernel(**inputs) -> np.ndarray:
    out, _ = _run(inputs, trace=False)
    return out


# revision 31
# speedup vs baseline: 1.0037x; 1.0037x over previous
"""Trainium2 Bass kernel for nn_CausalAttention (N=4096, 8 heads, DH=32).

Strategy: head-parallel across 8 NeuronCores (1 head per core).

Final (row-pipelined): the Scalar/ACT engine (exp over the causal score area,
~74K elems/partition) is the hard throughput floor (~62us/core), so the
kernel is organized to keep ACT busy from ~3us on and hide everything else
under it:
  - Software pipeline: DMA slice s -> project slice s -> process query-row s
    (scores + exp + PV for query block 512s..512s+512, k-tiles 0..4s+3).
    Row s only needs projections from slices <= s, so exp starts right
    after the first 1MB of input lands instead of after all projections.
  - Exp batching: scores grouped 3 k-tiles per PSUM tile [128, 1536]
    (double-buffered, 6 banks) -> 51 ACTIVATE calls instead of 72.
  - Score matmuls 4-way row-packed (K=32 quadrants of the PE array) via a
    kT4 layout: band u of partitions holds k-tile 4g+u; qT is 4x
    partition-replicated so all bands see the same queries.
  - All DMA on the Sync HWDGE queue (gpsimd software-DGE is ~100x slower;
    Scalar triggers would eat exp throughput). Inputs host-cast to bf16
    (halves HBM traffic); first k+q slice fused into one transfer; later
    slices in 1024-col interleaved chunks for descriptor efficiency.
  - 13 dummy matmuls at start warm the PE clock (HAM 1.2->2.4GHz)
    during the initial DMA wait.
  - Max-free softmax: P^T = exp(S / sqrt(32)); strict-causal 0/1 mask
    applied post-exp on the diagonal tiles (exp never overflows; the
    reference's -10000 masking underflows to exactly 0 in f32).
  - Softmax denominator via a ones column appended to V (PV lhsT [128, 33]);
    P^T and V in bf16 (f32 accumulate). Normalization: colsum -> reshape
    DMA -> reciprocal -> K=1 matmul broadcast -> one tensor_mul. Output
    stays in O^T layout [32, 4096] per core; host reshapes.
"""

import math

import numpy as np
import ml_dtypes

import concourse.bass as bass
import concourse.mybir as mybir
from concourse import bacc
from concourse.tile import TileContext
from concourse.bass_utils import run_bass_kernel_spmd

# Problem constants (hardcoded per harness contract).
B, CQ, CK, CH, NH, H, W = 1, 256, 256, 256, 8, 64, 64
DH = CH // NH            # 32
N = H * W                # 4096
QB = 512                 # queries per row (query block)
NQB = N // QB            # 8 rows
KT = 128                 # keys per k-tile
NKT = N // KT            # 32
EG = 3                   # k-tiles per exp/PSUM group (3 banks per tile)
SCALE = 1.0 / math.sqrt(DH)

F32 = mybir.dt.float32
F32R = mybir.dt.float32r
BF16 = mybir.dt.bfloat16

_CACHED_NC = None


def _build():
    nc = bacc.Bacc("TRN2", target_bir_lowering=False, debug=False, num_devices=1)

    qin_d = nc.dram_tensor("qin", [CQ, N], BF16, kind="ExternalInput")
    kin_d = nc.dram_tensor("kin", [CK, N], BF16, kind="ExternalInput")
    # fused first slice: kq0[(c p), t*512+n] = (kin if t==0 else qin)[:, n]
    kq0_d = nc.dram_tensor("kq0", [CQ, 2 * QB], BF16, kind="ExternalInput")
    wq_d = nc.dram_tensor("wqt", [CQ, 128], BF16, kind="ExternalInput")
    wk_d = nc.dram_tensor("wkt", [CK, 128], BF16, kind="ExternalInput")
    wv_d = nc.dram_tensor("wvt", [CK, DH], BF16, kind="ExternalInput")
    # bias_all[:, 0]=bq, [:, 1]=bk, [:, 2:2+4*DH]=bv (4x tiled)
    ball_d = nc.dram_tensor("ball", [128, 2 + 4 * DH], F32, kind="ExternalInput")
    out_d = nc.dram_tensor("out", [DH, N], F32, kind="ExternalOutput")

    # Strict-causal mask window: tm[kk, j] = 1.0 iff kk < j - 384; the
    # [*, 384:512] slice gives mask[kk, qq] = (kk < qq) for the 128-wide
    # diagonal window.
    tm_np = (np.arange(128)[:, None] < (np.arange(512)[None, :] - 384)).astype(
        ml_dtypes.bfloat16
    )
    tm_d = nc.inline_tensor(tm_np, name="tmask")
    ones_d = nc.inline_tensor(np.ones((1, DH), dtype=np.float32), name="onesd")

    with TileContext(nc) as tc:
        with (
            tc.tile_pool(name="constp", bufs=1) as constp,
            tc.tile_pool(name="bigp", bufs=1) as bigp,
            tc.tile_pool(name="workp", bufs=4) as workp,
            tc.tile_pool(name="spool", bufs=2, space="PSUM") as spool,
            tc.tile_pool(name="xpool", bufs=1, space="PSUM") as xpool,
            tc.tile_pool(name="opool", bufs=1, space="PSUM") as opool,
        ):
            # ---- PE warmup: dummy matmuls during the initial DMA wait so
            # HAM un-throttles the PE clock (1.2->2.4 GHz) before real work
            warm_sb = constp.tile([128, 512], BF16, name="warm_sb")
            nc.vector.memset(warm_sb[:], 0.0)
            warm_ps = xpool.tile([128, 512], F32, name="warm_ps", tag="x")
            for _ in range(13):
                nc.tensor.matmul(
                    warm_ps[:], warm_sb[:, 0:128], warm_sb[:], start=True, stop=True
                )

            # ---- weights/constants + inputs, all on the Sync HWDGE queue,
            # ordered so the projection chain unblocks earliest
            kin_sb = bigp.tile([128, 2, N], BF16, name="kin_sb")
            qin_sb = bigp.tile([128, 2, N], BF16, name="qin_sb")
            kin_ap = kin_d.ap().rearrange("(c p) n -> p c n", p=128)
            qin_ap = qin_d.ap().rearrange("(c p) n -> p c n", p=128)

            # fused slice 0 of kin+qin in one transfer (biggest first-exp win)
            sl0 = slice(0, QB)
            kq0_ap = kq0_d.ap().rearrange("(c p) (t n) -> p c t n", p=128, t=2)
            nc.sync.dma_start(kin_sb[:, :, sl0], kq0_ap[:, :, 0, :])
            nc.sync.dma_start(qin_sb[:, :, sl0], kq0_ap[:, :, 1, :])
            wk_sb = constp.tile([128, 2, 128], BF16, name="wk_sb")
            nc.sync.dma_start(
                wk_sb[:], wk_d.ap().rearrange("(c p) m -> p c m", p=128)
            )
            wq_sb = constp.tile([128, 2, 128], BF16, name="wq_sb")
            nc.sync.dma_start(
                wq_sb[:], wq_d.ap().rearrange("(c p) m -> p c m", p=128)
            )
            ball_sb = constp.tile([128, 2 + 4 * DH], F32, name="ball_sb")
            bq_sb = ball_sb[:, 0:1]
            bk_sb = ball_sb[:, 1:2]
            bv_sb = ball_sb[:, 2 : 2 + 4 * DH].rearrange("p (a b) -> p a b", a=4)
            # only bq/bk gate the first quad; bv + wv can follow slice 1
            nc.sync.dma_start(ball_sb[:, 0:2], ball_d.ap()[:, 0:2])
            # interleave kin/qin chunks so early rows unblock soonest
            nc.sync.dma_start(kin_sb[:, :, 512:1024], kin_ap[:, :, 512:1024])
            nc.sync.dma_start(qin_sb[:, :, 512:1024], qin_ap[:, :, 512:1024])
            nc.sync.dma_start(
                ball_sb[:, 2 : 2 + 4 * DH], ball_d.ap()[:, 2 : 2 + 4 * DH]
            )
            wv_sb = constp.tile([128, 2, DH], BF16, name="wv_sb")
            nc.sync.dma_start(
                wv_sb[:], wv_d.ap().rearrange("(c p) m -> p c m", p=128)
            )
            nc.sync.dma_start(kin_sb[:, :, 1024:1536], kin_ap[:, :, 1024:1536])
            nc.sync.dma_start(qin_sb[:, :, 1024:1536], qin_ap[:, :, 1024:1536])
            tm_sb = constp.tile([128, 512], BF16, name="tm_sb")
            nc.sync.dma_start(tm_sb[:], tm_d.ap())
            nc.sync.dma_start(kin_sb[:, :, 1536:2560], kin_ap[:, :, 1536:2560])
            nc.sync.dma_start(qin_sb[:, :, 1536:2560], qin_ap[:, :, 1536:2560])
            ones_sb = constp.tile([1, DH], F32, name="ones_sb")
            nc.sync.dma_start(ones_sb[:], ones_d.ap())
            nc.sync.dma_start(kin_sb[:, :, 2560:4096], kin_ap[:, :, 2560:4096])
            nc.sync.dma_start(qin_sb[:, :, 2560:4096], qin_ap[:, :, 2560:4096])

            # ---- persistent projected tensors ----
            # kT4[32u+d, 128g+kk] = k^T[d, 128*(4g+u)+kk]  (4-way row packing)
            kT4 = bigp.tile([128, (NKT // 4) * 128], BF16, name="kT4")
            # qT[32u+d, q] = q^T[d, q] for u=0..3 (4x replicated on partitions)
            qT = bigp.tile([128, N], BF16, name="qT")
            # v_all[kk, t, :DH] = v[128t+kk, :]; col DH is the ones column
            v_all = bigp.tile([128, NKT, 48], BF16, name="v_all")
            nc.vector.memset(v_all[:, :, DH : DH + 1], 1.0)

            def emit_kq_proj(s):
                """Project slice s: k-tiles 4s..4s+3 into kT4 + qT block s."""
                ksl = slice(QB * s, QB * (s + 1))
                pj = xpool.tile([128, 512], F32, name="pj", tag="x")
                for ch in range(2):
                    nc.tensor.matmul(
                        pj[:],
                        wk_sb[:, ch, :],
                        kin_sb[:, ch, ksl],
                        start=(ch == 0),
                        stop=(ch == 1),
                    )
                for ci in range(4):
                    # k-tile j = 4s+ci -> band u=ci, column group g=s.
                    # Early slices: do the adds on ScalarE (idle pre-exp,
                    # identity is in the exp table set) so they run in
                    # parallel with the qT add on DVE.
                    dst = kT4[32 * ci : 32 * ci + 32, 128 * s : 128 * s + 128]
                    srcp = pj[32 * ci : 32 * ci + 32, 128 * ci : 128 * ci + 128]
                    bias = bk_sb[32 * ci : 32 * ci + 32, :]
                    if s <= 1:
                        nc.scalar.activation(
                            dst,
                            srcp,
                            mybir.ActivationFunctionType.Identity,
                            bias=bias,
                        )
                    else:
                        nc.vector.tensor_scalar_add(dst, srcp, bias)
                pj = xpool.tile([128, 512], F32, name="pj", tag="x")
                for ch in range(2):
                    nc.tensor.matmul(
                        pj[:],
                        wq_sb[:, ch, :],
                        qin_sb[:, ch, ksl],
                        start=(ch == 0),
                        stop=(ch == 1),
                    )
                nc.vector.tensor_scalar_add(qT[:, ksl], pj[:], bq_sb[:])

            def emit_v_proj(s):
                """Project slice s into v_all tiles 4s..4s+3 (one shared
                psum bank for all 4 tiles, one combined bias-add)."""
                pj = xpool.tile([128, 4, DH], F32, name="pj", tag="x")
                for ti in range(4):
                    t = 4 * s + ti
                    nsl = slice(128 * t, 128 * (t + 1))
                    for ch in range(2):
                        nc.tensor.matmul(
                            pj[:, ti, :],
                            kin_sb[:, ch, nsl],
                            wv_sb[:, ch, :],
                            start=(ch == 0),
                            stop=(ch == 1),
                        )
                nc.vector.tensor_add(
                    v_all[:, 4 * s : 4 * s + 4, 0:DH], pj[:], bv_sb[:]
                )

            stage_q = []  # deferred tail stages, advanced one per exp group

            def tail_a(st):
                o_ps = st["o_ps"]
                o_sb = workp.tile([DH, 512], F32, name="o_sb")
                nc.vector.tensor_copy(o_sb[:], o_ps[0:DH, :])
                cs_sb = workp.tile([1, 512], F32, name="cs_sb")
                # +1e-30 keeps q=0 (fully masked row) at 0 instead of NaN
                nc.vector.tensor_scalar_add(cs_sb[:], o_ps[DH : DH + 1, :], 1e-30)
                st.update(o_sb=o_sb, cs_sb=cs_sb)

            def tail_b(st):
                # ~18-bit reciprocal, no DMA round trips
                csr = workp.tile([1, 512], F32, name="csr")
                nc.vector.reciprocal_approx_fast(csr[:], st["cs_sb"][:])
                st.update(csr=csr)

            def tail_c(st):
                qb = st["qb"]
                rep_ps = xpool.tile([DH, 512], F32, name="rep_ps", tag="x")
                nc.tensor.matmul(
                    rep_ps[:], ones_sb[:], st["csr"][:], start=True, stop=True
                )
                out_sb = workp.tile([DH, 512], F32, name="out_sb")
                nc.vector.tensor_mul(out_sb[:], st["o_sb"][:], rep_ps[:])
                nc.sync.dma_start(
                    out_d.ap()[:, 512 * qb : 512 * (qb + 1)], out_sb[:]
                )

            # Global deferred queues: PV work for a finished exp group is
            # flushed lazily (possibly during the NEXT row's quads) so the
            # in-order PE stream never makes ACT wait at a row boundary.
            pends = []  # (row_state, g, nsub, p_sb, row_final)

            def flush_one():
                st, g, nsub, p_sb, row_final = pends.pop(0)
                nlast = st["nkt"] - 1
                for u in range(nsub):
                    j = EG * g + u
                    nc.tensor.matmul(
                        st["o_ps"][0 : DH + 1, :],
                        v_all[:, j, 0 : DH + 1],
                        p_sb[:, 512 * u : 512 * (u + 1)],
                        start=st["first"][0],
                        stop=(j == nlast),
                        skip_group_check=True,
                    )
                    st["first"][0] = False
                if row_final:
                    tail_a(st)
                    stage_q.append(lambda st=st: tail_b(st))
                    stage_q.append(lambda: None)
                    stage_q.append(lambda: None)
                    stage_q.append(lambda st=st: tail_c(st))

            def emit_row(s):
                """Queries 512s..512s+512 vs k-tiles 0..4s+3."""
                nkt_q = 4 * (s + 1)
                ngr = (nkt_q + EG - 1) // EG
                o_ps = opool.tile([128, 512], F32, name="o_ps", tag="o")
                st = {"qb": s, "o_ps": o_ps, "first": [True, True], "nkt": nkt_q}
                groups = []

                def finish_group(g, nsub):
                    """Exp + mask + queue PV for a fully-written group."""
                    s_ps = groups[g][0]
                    p_sb = workp.tile(
                        [128, EG * 512], BF16, name="p_sb", tag="p", bufs=8
                    )
                    nc.scalar.activation(
                        p_sb[:, 0 : 512 * nsub],
                        s_ps[:, 0 : 512 * nsub],
                        mybir.ActivationFunctionType.Exp,
                        scale=SCALE,
                    )
                    for u in range(nsub):
                        j = EG * g + u
                        o = 128 * j - 512 * s
                        if o > 0:  # zero the fully-masked prefix (stale exp)
                            nc.vector.memset(p_sb[:, 512 * u : 512 * u + o], 0.0)
                        if o >= 0:  # strict-causal mask on the diagonal window
                            nc.vector.tensor_mul(
                                p_sb[:, 512 * u + o : 512 * u + o + 128],
                                p_sb[:, 512 * u + o : 512 * u + o + 128],
                                tm_sb[:, 384:512],
                            )
                    pends.append((st, g, nsub, p_sb, g == ngr - 1))
                    # near a row boundary let PV queue up so the next row's
                    # first quad isn't stuck behind it in the PE stream
                    # (final row excepted: drain fast so the tail starts)
                    cap = 4 if (g >= ngr - 2 and s < NQB - 1) else 2
                    if len(pends) > cap:
                        flush_one()
                    if stage_q:
                        stage_q.pop(0)()

                ndone = 0  # exp groups emitted
                for m in range(s + 1):  # score quads
                    # ensure psum groups for tiles 4m..4m+3 exist
                    while len(groups) * EG < 4 * (m + 1):
                        g = len(groups)
                        nsub = min(EG, nkt_q - EG * g)
                        t = spool.tile([128, EG * 512], F32, name="s_ps", tag="s")
                        groups.append((t, nsub))
                    for u in range(4):
                        j = 4 * m + u
                        g, slot = j // EG, j % EG
                        o = max(0, 128 * j - 512 * s)
                        nc.tensor.matmul(
                            groups[g][0][:, 512 * slot + o : 512 * (slot + 1)],
                            kT4[32 * u : 32 * u + 32, 128 * m : 128 * m + 128],
                            qT[32 * u : 32 * u + 32, 512 * s + o : 512 * (s + 1)],
                            start=True,
                            stop=True,
                            tile_position=(32 * u, 0),
                        )
                    # next slice's projections early, under this row's exps
                    if s == 0 and m == 0:
                        emit_v_proj(0)
                    if s + 1 < NQB:
                        if m == 0:
                            emit_kq_proj(s + 1)
                        if m == min(1, s):
                            emit_v_proj(s + 1)
                    # emit exp for every fully-written group
                    while (ndone + 1) * EG <= 4 * (m + 1) or (
                        m == s and ndone < ngr
                    ):
                        finish_group(ndone, groups[ndone][1])
                        ndone += 1

            emit_kq_proj(0)
            for s in range(NQB):
                emit_row(s)
            while pends:
                flush_one()
            while stage_q:
                stage_q.pop(0)()

    nc.finalize()
    return nc


def _get_nc():
    global _CACHED_NC
    if _CACHED_NC is None:
        _CACHED_NC = _build()
    return _CACHED_NC


def _prep_in_maps(inputs):
    f = lambda a: np.ascontiguousarray(np.asarray(a, dtype=np.float32))
    query = f(inputs["query"]).reshape(CQ, N)
    key_feat = f(inputs["key_feat"]).reshape(CK, N)

    def wnorm(v, g):
        v = f(v)
        g = f(g)
        return g[:, None] * v / np.linalg.norm(v, axis=1, keepdims=True)

    wq = wnorm(inputs["vq"], inputs["gq"])
    wk = wnorm(inputs["vk"], inputs["gk"])
    wv = wnorm(inputs["vv"], inputs["gv"])
    bq, bk, bv = f(inputs["bq"]), f(inputs["bk"]), f(inputs["bv"])

    bf = lambda a: np.ascontiguousarray(a).astype(ml_dtypes.bfloat16)
    query_bf = bf(query)
    key_bf = bf(key_feat)
    kq0 = np.concatenate([key_feat[:, :QB], query[:, :QB]], axis=1)
    in_maps = []
    for c in range(NH):
        rows = slice(DH * c, DH * (c + 1))
        in_maps.append(
            {
                "qin": query_bf,
                "kin": key_bf,
                "kq0": bf(kq0),
                "wqt": bf(np.tile(wq[rows].T, (1, 4))),
                "wkt": bf(np.tile(wk[rows].T, (1, 4))),
                "wvt": bf(wv[rows].T),
                "ball": np.ascontiguousarray(
                    np.concatenate(
                        [
                            np.tile(bq[rows], 4)[:, None],
                            np.tile(bk[rows], 4)[:, None],
                            np.tile(np.tile(bv[rows], 4)[None, :], (128, 1)),
                        ],
                        axis=1,
                    )
                ),
            }
        )
    return in_maps


def _run(inputs, trace=False, **kwargs):
    nc = _get_nc()
    in_maps = _prep_in_maps(inputs)
    res = None
    for attempt in range(3):
        try:
            res = run_bass_kernel_spmd(
                nc, in_maps, core_ids=list(range(NH)), trace=trace, **kwargs
            )
            break
        except Exception:
            if attempt == 2:
                raise

    out = np.empty((B, CH, H, W), dtype=np.float32)
    for c in range(NH):
        oc = res.results[c]["out"]  # [DH, N] (O^T layout)
        out[0, DH * c : DH * (c + 1)] = oc.reshape(DH, H, W)
    return out, res


def kernel(**inputs) -> np.ndarray:
    out, _ = _run(inputs, trace=False)
    return out
